# revision 14
# baseline (speedup 1.0000x reference)
"""Trainium2 Bass kernel for CustomFlashAttention (B=2, S=2048, D=2048, H=16).

Sharding over 8 NeuronCores: core c handles batch b=c//4 and head-group
hg=c%4 (4 heads of 128 dims = feature cols [hg*512,(hg+1)*512)).
Per core: QKV projections for its cols, causal flash attention for its 4
heads, partial output projection; host sums the 4 partials per batch.

All matmul operands are bf16 (same 1 cycle/row PE rate as fp32r at
N>=256, but faster weight loads, half the DMA bytes, and full-rate
narrow matmuls on causal-diagonal tiles). The whole kernel is one Tile
scope -- projections for token block tb+1 and the output projection for
block qb are emitted in the same window as attention for block qb, so
the out-of-order Tile scheduler fills the PE with projection matmuls
while the Scalar engine runs exp.

Consecutive matmuls never accumulate into the same PSUM bank (pairs of
accumulators are interleaved; PV(h) interleaves with scores(h+1)) --
back-to-back same-bank matmuls cost ~454ns instead of ~225ns because
the drain can't overlap the next fill.

Causal masking: exp runs on raw scores (no overflow possible at this
scale); the 128-wide diagonal staircase is zeroed afterwards by an
upper-triangle multiply. Softmax skips max-subtraction (scores ~N(0,1))
and defers normalization to the attention output. Row sums: heads 0-2
accumulate exp tiles on the Vector engine (one all-ones matmul at the
end); head 3 uses per-tile all-ones matmuls on the PE, where they
alternate banks with PV.
"""

import os
import numpy as np
import ml_dtypes

import concourse.bacc as bacc
import concourse.mybir as mybir
import concourse.tile as tile
from concourse.bass_utils import run_bass_kernel_spmd

B = 2
S = 2048
D = 2048
H_PER_CORE = 4
DC = 512          # feature cols per core (4 heads * 128)
HD = 128          # head dim
P = 128
TB = 512          # token block
N_TB = S // TB    # 4
N_KT = S // P     # 16 (128-wide k/token tiles)
FP32 = mybir.dt.float32
FP32R = mybir.dt.float32r
BF16 = mybir.dt.bfloat16

LAST_RESULTS = None  # BassKernelResults from the most recent run (for test.py)


def build_bass(causal: bool):
    nc = bacc.Bacc(None, target_bir_lowering=False, debug=False)

    xT_d = nc.dram_tensor("xT", [D, S], BF16, kind="ExternalInput")
    wqT_d = nc.dram_tensor("wqT", [D, DC], BF16, kind="ExternalInput")
    wkT_d = nc.dram_tensor("wkT", [D, DC], BF16, kind="ExternalInput")
    wvT_d = nc.dram_tensor("wvT", [D, DC], BF16, kind="ExternalInput")
    woT_d = nc.dram_tensor("woT", [DC, D], BF16, kind="ExternalInput")
    ident_d = nc.dram_tensor("ident", [P, P], BF16, kind="ExternalInput")
    trineg_d = nc.dram_tensor("trineg", [P, P], BF16, kind="ExternalInput")
    out_d = nc.dram_tensor("out", [S, D], FP32, kind="ExternalOutput")

    x_r = xT_d.rearrange("(k p) t -> p k t", p=P)     # [128, 16, 2048]
    wq_r = wqT_d.rearrange("(k p) m -> p k m", p=P)   # [128, 16, 512]
    wk_r = wkT_d.rearrange("(k p) m -> p k m", p=P)
    wv_r = wvT_d.rearrange("(k p) m -> p k m", p=P)
    wo_r = woT_d.rearrange("(h p) n -> p h n", p=P)   # [128, 4, 2048]

    with tile.TileContext(nc) as tc:
        with tc.tile_pool(name="persist", bufs=1) as persist, \
             tc.tile_pool(name="xt", bufs=2) as xtpool, \
             tc.tile_pool(name="pt", bufs=8) as ptpool, \
             tc.tile_pool(name="pa", bufs=2) as papool, \
             tc.tile_pool(name="ot", bufs=8) as otpool, \
             tc.tile_pool(name="rc", bufs=2) as rcpool, \
             tc.tile_pool(name="ob", bufs=4) as obpool, \
             tc.tile_pool(name="pj", bufs=2, space="PSUM") as pjp, \
             tc.tile_pool(name="po", bufs=2, space="PSUM") as pop, \
             tc.tile_pool(name="pss", bufs=2, space="PSUM") as pssp, \
             tc.tile_pool(name="ppv", bufs=1, space="PSUM") as ppvp, \
             tc.tile_pool(name="pn", bufs=1, space="PSUM") as pnp:

            # persistent activations and weights, all bf16
            qt_s = persist.tile([P, H_PER_CORE, S], BF16, tag="qt")
            kt_s = persist.tile([P, H_PER_CORE, S], BF16, tag="kt")
            v_s = persist.tile([P, N_KT, DC], BF16, tag="v")
            wq_s = persist.tile([P, N_KT, DC], BF16, tag="wq")
            wk_s = persist.tile([P, N_KT, DC], BF16, tag="wk")
            wv_s = persist.tile([P, N_KT, DC], BF16, tag="wv")
            wo_s = persist.tile([P, H_PER_CORE, D], BF16, tag="wo")
            ones_f = persist.tile([P, P], FP32, tag="onesf")
            ones_s = persist.tile([P, P], BF16, tag="ones")
            ones_r = persist.tile([P, P], FP32R, tag="onesr")
            nc.vector.memset(ones_f[:], 1.0)
            nc.vector.tensor_copy(out=ones_s[:], in_=ones_f[:])
            nc.vector.tensor_copy(out=ones_r[:], in_=ones_f[:])
            # prologue DMAs: wq and x(tb0) interleaved in chunks (first
            # chunks small) so the first projection matmuls start early
            xt0 = xtpool.tile([P, N_KT, TB], BF16, tag="xt", name="xt0")
            chunks = [(0, 2), (2, 4), (4, 16)]
            for ci, (lo, hi) in enumerate(chunks):
                nc.sync.dma_start(wq_s[:, lo:hi, :], wq_r[:, lo:hi, :])
                nc.sync.dma_start(xt0[:, lo:hi, :], x_r[:, lo:hi, 0:TB])
            nc.sync.dma_start(wv_s[:], wv_r[:])
            nc.sync.dma_start(wk_s[:], wk_r[:])
            if causal:
                ident_s = persist.tile([P, P], BF16, tag="ident")
                trineg_s = persist.tile([P, P], BF16, tag="trineg")
                nc.sync.dma_start(ident_s[:], ident_d[:])
                nc.sync.dma_start(trineg_s[:], trineg_d[:])

            def proj_chunk(tb, xt):
                """Q, V, K projections for token block tb (512 tokens).

                Two accumulators run interleaved so consecutive matmuls
                never target the same PSUM bank."""
                t0 = tb * TB
                for hp in range(2):
                    ha, hb = 2 * hp, 2 * hp + 1
                    pa = pjp.tile([P, TB], FP32, tag="pj",
                                  name=f"pjq{tb}_{ha}")
                    pb = pjp.tile([P, TB], FP32, tag="pj",
                                  name=f"pjq{tb}_{hb}")
                    for kt in range(N_KT):
                        st, sp = kt == 0, kt == N_KT - 1
                        nc.tensor.matmul(
                            pa[:], wq_s[:, kt, ha * HD:(ha + 1) * HD],
                            xt[:, kt, :], start=st, stop=sp)
                        nc.tensor.matmul(
                            pb[:], wq_s[:, kt, hb * HD:(hb + 1) * HD],
                            xt[:, kt, :], start=st, stop=sp)
                    nc.vector.tensor_copy(out=qt_s[:, ha, t0:t0 + TB],
                                          in_=pa[:])
                    nc.vector.tensor_copy(out=qt_s[:, hb, t0:t0 + TB],
                                          in_=pb[:])
                for tp in range(2):
                    ta, tb_ = 2 * tp, 2 * tp + 1
                    pa = pjp.tile([P, TB], FP32, tag="pj",
                                  name=f"pjv{tb}_{ta}")
                    pb = pjp.tile([P, TB], FP32, tag="pj",
                                  name=f"pjv{tb}_{tb_}")
                    for kt in range(N_KT):
                        st, sp = kt == 0, kt == N_KT - 1
                        nc.tensor.matmul(
                            pa[:], xt[:, kt, ta * P:(ta + 1) * P],
                            wv_s[:, kt, :], start=st, stop=sp)
                        nc.tensor.matmul(
                            pb[:], xt[:, kt, tb_ * P:(tb_ + 1) * P],
                            wv_s[:, kt, :], start=st, stop=sp)
                    nc.vector.tensor_copy(out=v_s[:, tb * 4 + ta, :],
                                          in_=pa[:])
                    nc.vector.tensor_copy(out=v_s[:, tb * 4 + tb_, :],
                                          in_=pb[:])
                for hp in range(2):
                    ha, hb = 2 * hp, 2 * hp + 1
                    pa = pjp.tile([P, TB], FP32, tag="pj",
                                  name=f"pjk{tb}_{ha}")
                    pb = pjp.tile([P, TB], FP32, tag="pj",
                                  name=f"pjk{tb}_{hb}")
                    for kt in range(N_KT):
                        st, sp = kt == 0, kt == N_KT - 1
                        nc.tensor.matmul(
                            pa[:], wk_s[:, kt, ha * HD:(ha + 1) * HD],
                            xt[:, kt, :], start=st, stop=sp)
                        nc.tensor.matmul(
                            pb[:], wk_s[:, kt, hb * HD:(hb + 1) * HD],
                            xt[:, kt, :], start=st, stop=sp)
                    nc.vector.tensor_copy(out=kt_s[:, ha, t0:t0 + TB],
                                          in_=pa[:])
                    nc.vector.tensor_copy(out=kt_s[:, hb, t0:t0 + TB],
                                          in_=pb[:])

            proj_chunk(0, xt0)
            nc.sync.dma_start(wo_s[:], wo_r[:])

            xt_tiles = {0: xt0}
            for qb in range(N_TB):
                # prefetch x for the next token block
                if qb + 1 < N_TB:
                    xt_n = xtpool.tile([P, N_KT, TB], BF16, tag="xt",
                                       name=f"xt{qb + 1}")
                    nc.sync.dma_start(
                        xt_n[:], x_r[:, :, (qb + 1) * TB:(qb + 2) * TB])
                    xt_tiles[qb + 1] = xt_n

                nkt = 4 * qb + 4 if causal else N_KT

                def emit_score(h, kt):
                    """scores^T -> exp -> P^T tile [key 128, q <=512].

                    Diagonal tiles accumulate a constant -30000 strict-
                    lower-triangle into the 128-wide staircase region of
                    the scores PSUM (a tiny N=128 matmul) so exp yields
                    exact zeros in masked lanes -- no post-exp masking."""
                    diag = causal and kt >= 4 * qb
                    s0 = (kt - 4 * qb) * P if diag else 0
                    ps_s = pssp.tile([P, TB], FP32, tag="s",
                                     name=f"s{qb}_{h}_{kt}")
                    nc.tensor.matmul(
                        ps_s[:, s0:],
                        kt_s[:, h, kt * P:(kt + 1) * P],
                        qt_s[:, h, qb * TB + s0:(qb + 1) * TB],
                        start=True, stop=not diag)
                    if diag:
                        nc.tensor.matmul(
                            ps_s[:, s0:s0 + P], ident_s[:], trineg_s[:],
                            start=False, stop=True)
                    ptile = ptpool.tile([P, TB], BF16, tag="p",
                                        name=f"p{qb}_{h}_{kt}")
                    nc.scalar.activation(
                        ptile[:, s0:], ps_s[:, s0:],
                        mybir.ActivationFunctionType.Exp)
                    return ptile, s0

                pts = {0: [emit_score(0, kt) for kt in range(nkt)]}
                ot_tiles = []
                for h in range(H_PER_CORE):
                    pe_rowsum = h == H_PER_CORE - 1
                    ps_o = ppvp.tile([P, TB], FP32, tag="pv",
                                     name=f"o{qb}_{h}")
                    if pe_rowsum:
                        ps_n = pnp.tile([P, TB], FP32, tag="n",
                                        name=f"n{qb}_{h}")
                    else:
                        pacc = papool.tile([P, TB], FP32R, tag="pa",
                                           name=f"pa{qb}_{h}")
                    if h + 1 < H_PER_CORE:
                        pts[h + 1] = []
                    for i in range(nkt):
                        ptile, s0 = pts[h][i]
                        first, last = i == 0, i == nkt - 1
                        nc.tensor.matmul(
                            ps_o[:, s0:],
                            v_s[:, i, h * HD:(h + 1) * HD],
                            ptile[:, s0:],
                            start=first, stop=last)
                        if pe_rowsum:
                            nc.tensor.matmul(
                                ps_n[:, s0:], ones_s[:], ptile[:, s0:],
                                start=first, stop=last)
                        else:
                            if first:
                                nc.vector.tensor_copy(out=pacc[:],
                                                      in_=ptile[:])
                            else:
                                nc.vector.tensor_tensor(
                                    pacc[:, s0:], pacc[:, s0:],
                                    ptile[:, s0:], mybir.AluOpType.add)
                        if h + 1 < H_PER_CORE:
                            # interleave next head's scores so PV matmuls
                            # never hit the PV bank back-to-back
                            pts[h + 1].append(emit_score(h + 1, i))
                    if not pe_rowsum:
                        ps_n = pnp.tile([P, TB], FP32, tag="n",
                                        name=f"n{qb}_{h}")
                        nc.tensor.matmul(ps_n[:], ones_r[:], pacc[:],
                                         start=True, stop=True)
                    recip = rcpool.tile([P, TB], FP32, tag="r",
                                        name=f"r{qb}_{h}")
                    nc.vector.reciprocal_approx_fast(out=recip[:],
                                                     in_=ps_n[:])
                    ot = otpool.tile([P, TB], BF16, tag="ot",
                                     name=f"ot{qb}_{h}")
                    nc.vector.tensor_tensor(
                        ot[:], ps_o[:], recip[:], mybir.AluOpType.mult)
                    ot_tiles.append(ot)

                # projections for the next token block (PE filler while
                # the Scalar engine works through exp)
                if qb + 1 < N_TB:
                    proj_chunk(qb + 1, xt_tiles[qb + 1])

                # output projection for this token block; two PSUM
                # accumulators interleaved
                # in the last stage the projection PSUM banks are free --
                # rotate the output projection over both pools
                last = qb == N_TB - 1
                for tt in range(4):
                    row0 = qb * TB + tt * P
                    for np_ in range(2):
                        na, nb = 2 * np_, 2 * np_ + 1
                        pool_a = pjp if last and tt % 2 else pop
                        tag_a = "pj" if last and tt % 2 else "po"
                        pa = pool_a.tile([P, TB], FP32, tag=tag_a,
                                         name=f"po{qb}_{tt}_{na}")
                        pb = pool_a.tile([P, TB], FP32, tag=tag_a,
                                         name=f"po{qb}_{tt}_{nb}")
                        for h in range(H_PER_CORE):
                            st, sp = h == 0, h == H_PER_CORE - 1
                            nc.tensor.matmul(
                                pa[:],
                                ot_tiles[h][:, tt * P:(tt + 1) * P],
                                wo_s[:, h, na * TB:(na + 1) * TB],
                                start=st, stop=sp)
                            nc.tensor.matmul(
                                pb[:],
                                ot_tiles[h][:, tt * P:(tt + 1) * P],
                                wo_s[:, h, nb * TB:(nb + 1) * TB],
                                start=st, stop=sp)
                        oa = obpool.tile([P, TB], FP32, tag="ob",
                                         name=f"ob{qb}_{tt}_{na}")
                        ob = obpool.tile([P, TB], FP32, tag="ob",
                                         name=f"ob{qb}_{tt}_{nb}")
                        if last:
                            # Scalar is idle once the final exps are done,
                            # and these copies sit after every exp in its
                            # queue -- no head-of-line risk
                            nc.scalar.copy(out=oa[:], in_=pa[:])
                            nc.scalar.copy(out=ob[:], in_=pb[:])
                        else:
                            nc.vector.tensor_copy(out=oa[:], in_=pa[:])
                            nc.vector.tensor_copy(out=ob[:], in_=pb[:])
                        nc.sync.dma_start(
                            out_d[row0:row0 + P, na * TB:(na + 1) * TB],
                            oa[:])
                        nc.sync.dma_start(
                            out_d[row0:row0 + P, nb * TB:(nb + 1) * TB],
                            ob[:])

    nc.compile()
    return nc


_BASS_CACHE = {}


def kernel(x, w_q, w_k, w_v, w_o, causal):
    global LAST_RESULTS
    x = np.asarray(x, dtype=np.float32)
    w_q = np.asarray(w_q, dtype=np.float32)
    w_k = np.asarray(w_k, dtype=np.float32)
    w_v = np.asarray(w_v, dtype=np.float32)
    w_o = np.asarray(w_o, dtype=np.float32)
    is_causal = bool(int(causal))

    if is_causal not in _BASS_CACHE:
        _BASS_CACHE[is_causal] = build_bass(is_causal)
    nc = _BASS_CACHE[is_causal]

    scale = np.float32(1.0 / np.sqrt(HD))
    # trineg[i, j] = -30000 where j < i (masked staircase lanes), else 0
    ident = np.eye(P, dtype=np.float32).astype(ml_dtypes.bfloat16)
    trineg = np.where(np.arange(P)[None, :] < np.arange(P)[:, None],
                      np.float32(-30000.0), np.float32(0.0)
                      ).astype(ml_dtypes.bfloat16)

    bf = ml_dtypes.bfloat16
    xT = [np.ascontiguousarray(x[b].T).astype(bf) for b in range(B)]
    in_maps = []
    for c in range(8):
        b, hg = divmod(c, 4)
        cols = slice(hg * DC, (hg + 1) * DC)
        in_maps.append({
            "xT": xT[b],
            "wqT": np.ascontiguousarray(w_q[cols, :].T * scale).astype(bf),
            "wkT": np.ascontiguousarray(w_k[cols, :].T).astype(bf),
            "wvT": np.ascontiguousarray(w_v[cols, :].T).astype(bf),
            "woT": np.ascontiguousarray(w_o[:, cols].T).astype(bf),
            "ident": ident,
            "trineg": trineg,
        })

    trace = bool(os.environ.get("KERNEL_TRACE"))
    try:
        res = run_bass_kernel_spmd(nc, in_maps, list(range(8)), trace=trace)
    except Exception:
        if not trace:
            raise
        res = run_bass_kernel_spmd(nc, in_maps, list(range(8)), trace=False)
    LAST_RESULTS = res

    out = np.zeros((B, S, D), dtype=np.float32)
    for c in range(8):
        b = c // 4
        out[b] += res.results[c]["out"]
    return out


# revision 20
# speedup vs baseline: 1.0144x; 1.0144x over previous
"""Trainium2 Bass kernel for CustomFlashAttention (B=2, S=2048, D=2048, H=16).

Sharding over 8 NeuronCores: core c handles batch b=c//4 and head-group
hg=c%4 (4 heads of 128 dims = feature cols [hg*512,(hg+1)*512)).
Per core: QKV projections for its cols, causal flash attention for its 4
heads, partial output projection; host sums the 4 partials per batch.

All matmul operands are bf16 (same 1 cycle/row PE rate as fp32r at
N>=256, but faster weight loads, half the DMA bytes, and full-rate
narrow matmuls on causal-diagonal tiles). The whole kernel is one Tile
scope -- projections for token block tb+1 and the output projection for
block qb are emitted in the same window as attention for block qb, so
the out-of-order Tile scheduler fills the PE with projection matmuls
while the Scalar engine runs exp.

Consecutive matmuls never accumulate into the same PSUM bank (pairs of
accumulators are interleaved; PV(h) interleaves with scores(h+1)) --
back-to-back same-bank matmuls cost ~454ns instead of ~225ns because
the drain can't overlap the next fill.

Causal masking: exp runs on raw scores (no overflow possible at this
scale); the 128-wide diagonal staircase is zeroed afterwards by an
upper-triangle multiply. Softmax skips max-subtraction (scores ~N(0,1))
and defers normalization to the attention output. Row sums: heads 0-2
accumulate exp tiles on the Vector engine (one all-ones matmul at the
end); head 3 uses per-tile all-ones matmuls on the PE, where they
alternate banks with PV.
"""

import os
import numpy as np
import ml_dtypes

import concourse.bacc as bacc
import concourse.mybir as mybir
import concourse.tile as tile
from concourse.bass_utils import run_bass_kernel_spmd

B = 2
S = 2048
D = 2048
H_PER_CORE = 4
DC = 512          # feature cols per core (4 heads * 128)
HD = 128          # head dim
P = 128
TB = 512          # token block
N_TB = S // TB    # 4
N_KT = S // P     # 16 (128-wide k/token tiles)
FP32 = mybir.dt.float32
FP32R = mybir.dt.float32r
BF16 = mybir.dt.bfloat16

LAST_RESULTS = None  # BassKernelResults from the most recent run (for test.py)


def build_bass(causal: bool):
    nc = bacc.Bacc(None, target_bir_lowering=False, debug=False)

    xT_d = nc.dram_tensor("xT", [D, S], BF16, kind="ExternalInput")
    wqT_d = nc.dram_tensor("wqT", [D, DC], BF16, kind="ExternalInput")
    wkT_d = nc.dram_tensor("wkT", [D, DC], BF16, kind="ExternalInput")
    wvT_d = nc.dram_tensor("wvT", [D, DC], BF16, kind="ExternalInput")
    woT_d = nc.dram_tensor("woT", [DC, D], BF16, kind="ExternalInput")
    tri_d = nc.dram_tensor("tri", [P, P], BF16, kind="ExternalInput")
    out_d = nc.dram_tensor("out", [S, D], FP32, kind="ExternalOutput")

    x_r = xT_d.rearrange("(k p) t -> p k t", p=P)     # [128, 16, 2048]
    wq_r = wqT_d.rearrange("(k p) m -> p k m", p=P)   # [128, 16, 512]
    wk_r = wkT_d.rearrange("(k p) m -> p k m", p=P)
    wv_r = wvT_d.rearrange("(k p) m -> p k m", p=P)
    wo_r = woT_d.rearrange("(h p) n -> p h n", p=P)   # [128, 4, 2048]

    with tile.TileContext(nc) as tc:
        with tc.tile_pool(name="persist", bufs=1) as persist, \
             tc.tile_pool(name="xt", bufs=2) as xtpool, \
             tc.tile_pool(name="pt", bufs=8) as ptpool, \
             tc.tile_pool(name="pa", bufs=2) as papool, \
             tc.tile_pool(name="ot", bufs=8) as otpool, \
             tc.tile_pool(name="rc", bufs=2) as rcpool, \
             tc.tile_pool(name="ob", bufs=4) as obpool, \
             tc.tile_pool(name="pj", bufs=2, space="PSUM") as pjp, \
             tc.tile_pool(name="po", bufs=2, space="PSUM") as pop, \
             tc.tile_pool(name="pss", bufs=2, space="PSUM") as pssp, \
             tc.tile_pool(name="ppv", bufs=1, space="PSUM") as ppvp, \
             tc.tile_pool(name="pn", bufs=1, space="PSUM") as pnp:

            # persistent activations and weights, all bf16
            qt_s = persist.tile([P, H_PER_CORE, S], BF16, tag="qt")
            kt_s = persist.tile([P, H_PER_CORE, S], BF16, tag="kt")
            v_s = persist.tile([P, N_KT, DC], BF16, tag="v")
            wq_s = persist.tile([P, N_KT, DC], BF16, tag="wq")
            wk_s = persist.tile([P, N_KT, DC], BF16, tag="wk")
            wv_s = persist.tile([P, N_KT, DC], BF16, tag="wv")
            wo_s = persist.tile([P, H_PER_CORE, D], BF16, tag="wo")
            ones_f = persist.tile([P, P], FP32, tag="onesf")
            ones_s = persist.tile([P, P], BF16, tag="ones")
            ones_r = persist.tile([P, P], FP32R, tag="onesr")
            nc.vector.memset(ones_f[:], 1.0)
            nc.vector.tensor_copy(out=ones_s[:], in_=ones_f[:])
            nc.vector.tensor_copy(out=ones_r[:], in_=ones_f[:])
            # prologue DMAs: wq and x(tb0) interleaved in chunks (first
            # chunks small) so the first projection matmuls start early
            xt0 = xtpool.tile([P, N_KT, TB], BF16, tag="xt", name="xt0")
            chunks = [(0, 1), (1, 2), (2, 4), (4, 16)]
            for lo, hi in chunks:
                nc.sync.dma_start(wk_s[:, lo:hi, :], wk_r[:, lo:hi, :])
                nc.sync.dma_start(xt0[:, lo:hi, :], x_r[:, lo:hi, 0:TB])
            nc.sync.dma_start(wq_s[:], wq_r[:])
            nc.sync.dma_start(wv_s[:], wv_r[:])
            if causal:
                tri_s = persist.tile([P, P], BF16, tag="tri")
                nc.sync.dma_start(tri_s[:], tri_d[:])

            def proj_chunk(tb, xt):
                """K, Q, V projections for token block tb (512 tokens).

                K first so attention scores for this block can start
                before the chunk completes. Two accumulators always run
                interleaved so consecutive matmuls never target the same
                PSUM bank."""
                t0 = tb * TB
                for w_s, dst_kind in ((wk_s, "k"), (wq_s, "q")):
                    dst = kt_s if dst_kind == "k" else qt_s
                    for hp in range(2):
                        ha, hb = 2 * hp, 2 * hp + 1
                        pa = pjp.tile([P, TB], FP32, tag="pj",
                                      name=f"pj{dst_kind}{tb}_{ha}")
                        pb = pjp.tile([P, TB], FP32, tag="pj",
                                      name=f"pj{dst_kind}{tb}_{hb}")
                        for kt in range(N_KT):
                            st, sp = kt == 0, kt == N_KT - 1
                            nc.tensor.matmul(
                                pa[:], w_s[:, kt, ha * HD:(ha + 1) * HD],
                                xt[:, kt, :], start=st, stop=sp)
                            nc.tensor.matmul(
                                pb[:], w_s[:, kt, hb * HD:(hb + 1) * HD],
                                xt[:, kt, :], start=st, stop=sp)
                        nc.vector.tensor_copy(out=dst[:, ha, t0:t0 + TB],
                                              in_=pa[:])
                        nc.vector.tensor_copy(out=dst[:, hb, t0:t0 + TB],
                                              in_=pb[:])
                for tp in range(2):
                    ta, tb_ = 2 * tp, 2 * tp + 1
                    pa = pjp.tile([P, TB], FP32, tag="pj",
                                  name=f"pjv{tb}_{ta}")
                    pb = pjp.tile([P, TB], FP32, tag="pj",
                                  name=f"pjv{tb}_{tb_}")
                    for kt in range(N_KT):
                        st, sp = kt == 0, kt == N_KT - 1
                        nc.tensor.matmul(
                            pa[:], xt[:, kt, ta * P:(ta + 1) * P],
                            wv_s[:, kt, :], start=st, stop=sp)
                        nc.tensor.matmul(
                            pb[:], xt[:, kt, tb_ * P:(tb_ + 1) * P],
                            wv_s[:, kt, :], start=st, stop=sp)
                    nc.vector.tensor_copy(out=v_s[:, tb * 4 + ta, :],
                                          in_=pa[:])
                    nc.vector.tensor_copy(out=v_s[:, tb * 4 + tb_, :],
                                          in_=pb[:])

            proj_chunk(0, xt0)
            nc.sync.dma_start(wo_s[:], wo_r[:])

            xt_tiles = {0: xt0}
            for qb in range(N_TB):
                # prefetch x for the next token block
                if qb + 1 < N_TB:
                    xt_n = xtpool.tile([P, N_KT, TB], BF16, tag="xt",
                                       name=f"xt{qb + 1}")
                    nc.sync.dma_start(
                        xt_n[:], x_r[:, :, (qb + 1) * TB:(qb + 2) * TB])
                    xt_tiles[qb + 1] = xt_n

                nkt = 4 * qb + 4 if causal else N_KT

                def emit_score(h, kt):
                    """scores^T -> exp -> P^T tile [key 128, q <=512].

                    exp runs on raw scores (no overflow possible at this
                    scale); the 128-wide diagonal staircase is zeroed
                    afterwards by an upper-triangle multiply on DVE."""
                    diag = causal and kt >= 4 * qb
                    s0 = (kt - 4 * qb) * P if diag else 0
                    ps_s = pssp.tile([P, TB], FP32, tag="s",
                                     name=f"s{qb}_{h}_{kt}")
                    nc.tensor.matmul(
                        ps_s[:, s0:],
                        kt_s[:, h, kt * P:(kt + 1) * P],
                        qt_s[:, h, qb * TB + s0:(qb + 1) * TB],
                        start=True, stop=True)
                    ptile = ptpool.tile([P, TB], BF16, tag="p",
                                        name=f"p{qb}_{h}_{kt}")
                    nc.scalar.activation(
                        ptile[:, s0:], ps_s[:, s0:],
                        mybir.ActivationFunctionType.Exp)
                    if diag:
                        nc.vector.tensor_tensor(
                            ptile[:, s0:s0 + P], ptile[:, s0:s0 + P],
                            tri_s[:], mybir.AluOpType.mult)
                    return ptile, s0

                pts = {0: [emit_score(0, kt) for kt in range(nkt)]}
                ot_tiles = []
                for h in range(H_PER_CORE):
                    pe_rowsum = h == H_PER_CORE - 1
                    ps_o = ppvp.tile([P, TB], FP32, tag="pv",
                                     name=f"o{qb}_{h}")
                    if pe_rowsum:
                        ps_n = pnp.tile([P, TB], FP32, tag="n",
                                        name=f"n{qb}_{h}")
                    else:
                        pacc = papool.tile([P, TB], FP32R, tag="pa",
                                           name=f"pa{qb}_{h}")
                    if h + 1 < H_PER_CORE:
                        pts[h + 1] = []
                    for i in range(nkt):
                        ptile, s0 = pts[h][i]
                        first, last = i == 0, i == nkt - 1
                        nc.tensor.matmul(
                            ps_o[:, s0:],
                            v_s[:, i, h * HD:(h + 1) * HD],
                            ptile[:, s0:],
                            start=first, stop=last)
                        if pe_rowsum:
                            nc.tensor.matmul(
                                ps_n[:, s0:], ones_s[:], ptile[:, s0:],
                                start=first, stop=last)
                        else:
                            if first:
                                nc.vector.tensor_copy(out=pacc[:],
                                                      in_=ptile[:])
                            else:
                                nc.vector.tensor_tensor(
                                    pacc[:, s0:], pacc[:, s0:],
                                    ptile[:, s0:], mybir.AluOpType.add)
                        if h + 1 < H_PER_CORE:
                            # interleave next head's scores so PV matmuls
                            # never hit the PV bank back-to-back
                            pts[h + 1].append(emit_score(h + 1, i))
                    if not pe_rowsum:
                        ps_n = pnp.tile([P, TB], FP32, tag="n",
                                        name=f"n{qb}_{h}")
                        nc.tensor.matmul(ps_n[:], ones_r[:], pacc[:],
                                         start=True, stop=True)
                    recip = rcpool.tile([P, TB], FP32, tag="r",
                                        name=f"r{qb}_{h}")
                    nc.vector.reciprocal_approx_fast(out=recip[:],
                                                     in_=ps_n[:])
                    ot = otpool.tile([P, TB], BF16, tag="ot",
                                     name=f"ot{qb}_{h}")
                    nc.vector.tensor_tensor(
                        ot[:], ps_o[:], recip[:], mybir.AluOpType.mult)
                    ot_tiles.append(ot)

                # projections for the next token block (PE filler while
                # the Scalar engine works through exp)
                if qb + 1 < N_TB:
                    proj_chunk(qb + 1, xt_tiles[qb + 1])

                # output projection for this token block; two PSUM
                # accumulators interleaved
                # in the last stage the projection PSUM banks are free --
                # rotate the output projection over both pools
                last = qb == N_TB - 1
                for tt in range(4):
                    row0 = qb * TB + tt * P
                    for np_ in range(2):
                        na, nb = 2 * np_, 2 * np_ + 1
                        pool_a = pjp if last and tt % 2 else pop
                        tag_a = "pj" if last and tt % 2 else "po"
                        pa = pool_a.tile([P, TB], FP32, tag=tag_a,
                                         name=f"po{qb}_{tt}_{na}")
                        pb = pool_a.tile([P, TB], FP32, tag=tag_a,
                                         name=f"po{qb}_{tt}_{nb}")
                        for h in range(H_PER_CORE):
                            st, sp = h == 0, h == H_PER_CORE - 1
                            nc.tensor.matmul(
                                pa[:],
                                ot_tiles[h][:, tt * P:(tt + 1) * P],
                                wo_s[:, h, na * TB:(na + 1) * TB],
                                start=st, stop=sp)
                            nc.tensor.matmul(
                                pb[:],
                                ot_tiles[h][:, tt * P:(tt + 1) * P],
                                wo_s[:, h, nb * TB:(nb + 1) * TB],
                                start=st, stop=sp)
                        oa = obpool.tile([P, TB], FP32, tag="ob",
                                         name=f"ob{qb}_{tt}_{na}")
                        ob = obpool.tile([P, TB], FP32, tag="ob",
                                         name=f"ob{qb}_{tt}_{nb}")
                        if last:
                            # Scalar is idle once the final exps are done,
                            # and these copies sit after every exp in its
                            # queue -- no head-of-line risk
                            nc.scalar.copy(out=oa[:], in_=pa[:])
                            nc.scalar.copy(out=ob[:], in_=pb[:])
                        else:
                            nc.vector.tensor_copy(out=oa[:], in_=pa[:])
                            nc.vector.tensor_copy(out=ob[:], in_=pb[:])
                        nc.sync.dma_start(
                            out_d[row0:row0 + P, na * TB:(na + 1) * TB],
                            oa[:])
                        nc.sync.dma_start(
                            out_d[row0:row0 + P, nb * TB:(nb + 1) * TB],
                            ob[:])

    nc.compile()
    return nc


_BASS_CACHE = {}


def kernel(x, w_q, w_k, w_v, w_o, causal):
    global LAST_RESULTS
    x = np.asarray(x, dtype=np.float32)
    w_q = np.asarray(w_q, dtype=np.float32)
    w_k = np.asarray(w_k, dtype=np.float32)
    w_v = np.asarray(w_v, dtype=np.float32)
    w_o = np.asarray(w_o, dtype=np.float32)
    is_causal = bool(int(causal))

    if is_causal not in _BASS_CACHE:
        _BASS_CACHE[is_causal] = build_bass(is_causal)
    nc = _BASS_CACHE[is_causal]

    scale = np.float32(1.0 / np.sqrt(HD))
    # tri[i, j] = 1 where j >= i (keeps key-partition i for staircase
    # column j), 0 otherwise
    tri = (np.arange(P)[None, :] >= np.arange(P)[:, None]
           ).astype(ml_dtypes.bfloat16)

    bf = ml_dtypes.bfloat16
    xT = [np.ascontiguousarray(x[b].T).astype(bf) for b in range(B)]
    in_maps = []
    for c in range(8):
        b, hg = divmod(c, 4)
        cols = slice(hg * DC, (hg + 1) * DC)
        in_maps.append({
            "xT": xT[b],
            "wqT": np.ascontiguousarray(w_q[cols, :].T * scale).astype(bf),
            "wkT": np.ascontiguousarray(w_k[cols, :].T).astype(bf),
            "wvT": np.ascontiguousarray(w_v[cols, :].T).astype(bf),
            "woT": np.ascontiguousarray(w_o[:, cols].T).astype(bf),
            "tri": tri,
        })

    trace = bool(os.environ.get("KERNEL_TRACE"))
    try:
        res = run_bass_kernel_spmd(nc, in_maps, list(range(8)), trace=trace)
    except Exception:
        if not trace:
            raise
        res = run_bass_kernel_spmd(nc, in_maps, list(range(8)), trace=False)
    LAST_RESULTS = res

    out = np.zeros((B, S, D), dtype=np.float32)
    for c in range(8):
        b = c // 4
        out[b] += res.results[c]["out"]
    return out
